# revision 1
# baseline (speedup 1.0000x reference)
"""LocalSelfAttention (window=7) Trainium2 Bass kernel.

Full inputs in, full output out. Sharding: 8 cores = batch(4) x seq-half(2),
each core handles 1024 tokens with a 3-token zero-padded halo on xs.

Math notes (exact rewrites of the reference):
- reference projects zero-PADDED xs patches, so out-of-range taps have
  k = b_ks, v = b_vs. Softmax over taps is invariant to the per-(t,h)
  constant q . b_ks, so the K bias drops entirely (padded taps then score 0,
  matching zero-padded halo @ w_ks with no bias).
- softmax weights sum to 1, so the V bias contributes exactly b_vs to o;
  it is folded on the host into b' = b_vs @ w_fc + b_fc.

Pipeline per core (feature-major activations, transposed on the HOST):
- QT/KT feature-major via matmul(lhsT=weight tile, rhs=xT), V token-major
  via matmul(lhsT=xsT slice, rhs=weight tile).
- attention in 9 chunks of 122 tokens (last chunk overlaps, recomputing a
  74-token stripe, so every window is exactly 128 wide and each PV matmul
  needs a single V partition-tile), processed in head PAIRS:
  scores for both heads land in one PSUM tile (122, 256), band-masked
  softmax with wide DVE/ACT ops, PE-transpose of the prob tile, one PV
  matmul per head accumulating both heads into one (128, 122) PSUM tile.
- FC + rank-1 bias matmul + residual + layernorm (split copy/add/reduce on
  DVE: a fused PSUM-source tensor_tensor_reduce crashes the exec unit).
"""

import sys

for _p in ("/opt/trn_rl_repo",):
    if _p not in sys.path:
        sys.path.insert(0, _p)

import numpy as np
import ml_dtypes

BF16 = ml_dtypes.bfloat16

H, DK, DV, D = 16, 64, 64, 1024
NEI = 3
TEMP = 8.0
EPS = 1e-5
B, S = 4, 2048
NCORES = 8
T = (B * S) // NCORES          # 1024 tokens per core
TH = T + 2 * NEI               # 1030 halo tokens
P = 128
NT = T // P                    # 8 fc-phase token chunks
ND = D // P                    # 8 feature chunks
CL = 96                        # attention chunk length (PE M must be x32)
CST = [96 * i for i in range(10)] + [928]          # chunk starts
TH2 = 1056                     # padded halo width (window reads up to 1056)
NEG = -30000.0

_CACHE = {}


def _build_program(apply_affine: bool):
    import concourse.bacc as bacc
    import concourse.tile as tile
    from concourse import mybir
    from contextlib import ExitStack

    f32 = mybir.dt.float32
    bf16 = mybir.dt.bfloat16
    Alu = mybir.AluOpType
    Act = mybir.ActivationFunctionType

    nc = bacc.Bacc(
        "TRN2", target_bir_lowering=False, debug=False, enable_asserts=False
    )

    def din(name, shape, dt_):
        return nc.dram_tensor(name, shape, dt_, kind="ExternalInput").ap()

    xq32 = din("xq32", (T, D), f32)      # residual (token-major, f32)
    xqT = din("xqT", (D, T), bf16)       # x^T (host-transposed)
    xsT = din("xsT", (D, TH), bf16)      # xs^T with halo (host-transposed)
    wq = din("wq", (D, D), bf16)
    wk = din("wk", (D, D), bf16)
    wv = din("wv", (D, D), bf16)
    wf = din("wf", (D, D), bf16)
    bq = din("bq", (P, ND), f32)         # b_qs laid out [p, ec]
    bpr = din("bpr", (1, D), bf16)       # b_vs @ w_fc + b_fc
    msk = din("msk", (CL, 2 * P), f32)   # band mask 0 / NEG, two head slots
    idn = din("idn", (P, P), bf16)       # identity for PE transpose
    ones = din("ones", (1, P), bf16)     # rank-1 bias helper
    if apply_affine:
        lng = din("lng", (1, D), f32)
        lnb = din("lnb", (1, D), f32)
    yo = nc.dram_tensor("yo", (T, D), f32, kind="ExternalOutput").ap()

    with tile.TileContext(nc) as tc, ExitStack() as ctx:
        consts = ctx.enter_context(tc.tile_pool(name="consts", bufs=1))
        big = ctx.enter_context(tc.tile_pool(name="big", bufs=1))
        wpool = ctx.enter_context(tc.tile_pool(name="wpool", bufs=2))
        xrpool = ctx.enter_context(tc.tile_pool(name="xrpool", bufs=3))
        work = ctx.enter_context(tc.tile_pool(name="work", bufs=3))
        lnpool = ctx.enter_context(tc.tile_pool(name="lnpool", bufs=2))
        small = ctx.enter_context(tc.tile_pool(name="small", bufs=4))
        psA = ctx.enter_context(tc.tile_pool(name="psA", bufs=3, space="PSUM"))
        psS = ctx.enter_context(tc.tile_pool(name="psS", bufs=1, space="PSUM"))
        psT = ctx.enter_context(tc.tile_pool(name="psT", bufs=2, space="PSUM"))
        psO = ctx.enter_context(tc.tile_pool(name="psO", bufs=1, space="PSUM"))

        # ---- constants ----
        msk_sb = consts.tile([CL, 2 * P], f32, tag="msk")
        nc.sync.dma_start(out=msk_sb, in_=msk)
        idn_sb = consts.tile([P, P], bf16, tag="idn")
        nc.sync.dma_start(out=idn_sb, in_=idn)
        bq_sb = consts.tile([P, ND], f32, tag="bq")
        nc.sync.dma_start(out=bq_sb, in_=bq)
        bpr_sb = consts.tile([1, D], bf16, tag="bpr")
        nc.sync.dma_start(out=bpr_sb, in_=bpr)
        ones_sb = consts.tile([1, P], bf16, tag="ones")
        nc.sync.dma_start(out=ones_sb, in_=ones)
        eps_sb = consts.tile([P, 1], f32, tag="eps")
        nc.vector.memset(eps_sb, EPS)
        if apply_affine:
            import concourse.bass as bass

            g_bc = consts.tile([P, D], f32, tag="g_bc")
            b_bc = consts.tile([P, D], f32, tag="b_bc")
            nc.sync.dma_start(
                out=g_bc,
                in_=bass.AP(tensor=lng.tensor, offset=lng.offset,
                            ap=[[0, P]] + list(lng.ap[1:])),
            )
            nc.sync.dma_start(
                out=b_bc,
                in_=bass.AP(tensor=lnb.tensor, offset=lnb.offset,
                            ap=[[0, P]] + list(lnb.ap[1:])),
            )

        # ---- transposed activations: straight row-slice loads ----
        xT_t = []
        xsT_t = []
        for dc in range(ND):
            t1 = big.tile([P, T], bf16, tag=f"xT{dc}", name=f"xT{dc}")
            nc.sync.dma_start(out=t1, in_=xqT[dc * P:(dc + 1) * P, :])
            xT_t.append(t1)
        for dc in range(ND):
            t2 = big.tile([P, TH2], bf16, tag=f"xsT{dc}", name=f"xsT{dc}")
            nc.sync.dma_start(out=t2[:, 0:TH], in_=xsT[dc * P:(dc + 1) * P, :])
            nc.vector.memset(t2[:, TH:TH2], 0.0)
            xsT_t.append(t2)

        def load_w(wap, tagp):
            tiles = []
            for dc in range(ND):
                wt = wpool.tile([P, D], bf16, tag=f"w{dc}", name=f"w_{tagp}{dc}")
                nc.sync.dma_start(out=wt, in_=wap[dc * P:(dc + 1) * P, :])
                tiles.append(wt)
            return tiles

        # ---- QT projection: (e, t) feature-major, bias via ACT evict ----
        wq_t = load_w(wq, "q")
        QT = [big.tile([P, T], bf16, tag=f"QT{ec}", name=f"QT{ec}")
              for ec in range(ND)]
        for ec in range(ND):
            psa = psA.tile([P, 512], f32, tag="psA", name="ps_qa")
            psb = psA.tile([P, 512], f32, tag="psA", name="ps_qb")
            for dc in range(ND):
                lt = wq_t[dc][:, ec * P:(ec + 1) * P]
                nc.tensor.matmul(psa, lhsT=lt, rhs=xT_t[dc][:, 0:512],
                                 start=(dc == 0), stop=(dc == ND - 1))
                nc.tensor.matmul(psb, lhsT=lt, rhs=xT_t[dc][:, 512:1024],
                                 start=(dc == 0), stop=(dc == ND - 1))
            nc.scalar.activation(out=QT[ec][:, 0:512], in_=psa,
                                 func=Act.Identity,
                                 bias=bq_sb[:, ec:ec + 1], scale=1.0)
            nc.scalar.activation(out=QT[ec][:, 512:1024], in_=psb,
                                 func=Act.Identity,
                                 bias=bq_sb[:, ec:ec + 1], scale=1.0)

        # ---- KT projection: (e, t_halo) feature-major, no bias ----
        wk_t = load_w(wk, "k")
        KT = [big.tile([P, TH2], bf16, tag=f"KT{ec}", name=f"KT{ec}")
              for ec in range(ND)]
        for ec in range(ND):
            psa = psA.tile([P, 512], f32, tag="psA", name="ps_ka")
            psb = psA.tile([P, 512], f32, tag="psA", name="ps_kb")
            for dc in range(ND):
                lt = wk_t[dc][:, ec * P:(ec + 1) * P]
                nc.tensor.matmul(psa, lhsT=lt, rhs=xsT_t[dc][:, 0:512],
                                 start=(dc == 0), stop=(dc == ND - 1))
                nc.tensor.matmul(psb, lhsT=lt, rhs=xsT_t[dc][:, 512:1024],
                                 start=(dc == 0), stop=(dc == ND - 1))
            nc.scalar.activation(out=KT[ec][:, 0:512], in_=psa, func=Act.Copy)
            nc.scalar.activation(out=KT[ec][:, 512:1024], in_=psb,
                                 func=Act.Copy)
        for ec in range(ND):  # halo tail (incl zero padding)
            pst = psA.tile([P, TH2 - T], f32, tag="psA", name="ps_kt")
            for dc in range(ND):
                nc.tensor.matmul(pst,
                                 lhsT=wk_t[dc][:, ec * P:(ec + 1) * P],
                                 rhs=xsT_t[dc][:, T:TH2],
                                 start=(dc == 0), stop=(dc == ND - 1))
            nc.vector.tensor_copy(KT[ec][:, T:TH2], pst)

        # ---- V projection: token-major (halo-rows, e); 9 chunk tiles ----
        wv_t = load_w(wv, "v")
        V = []
        for ci, s in enumerate(CST):
            vt = big.tile([P, D], bf16, tag=f"V{ci}", name=f"V{ci}")
            psa = psA.tile([P, 512], f32, tag="psA", name="ps_va")
            psb = psA.tile([P, 512], f32, tag="psA", name="ps_vb")
            for dc in range(ND):
                lt = xsT_t[dc][:, s:s + P]
                nc.tensor.matmul(psa, lhsT=lt, rhs=wv_t[dc][:, 0:512],
                                 start=(dc == 0), stop=(dc == ND - 1))
                nc.tensor.matmul(psb, lhsT=lt, rhs=wv_t[dc][:, 512:1024],
                                 start=(dc == 0), stop=(dc == ND - 1))
            nc.scalar.activation(out=vt[:, 0:512], in_=psa, func=Act.Copy)
            nc.scalar.activation(out=vt[:, 512:1024], in_=psb, func=Act.Copy)
            V.append(vt)

        # prefetch FC weights during attention
        wf_t = load_w(wf, "f")

        # ---- windowed attention: chunks of 96, head pairs ----
        OT = [big.tile([P, T], bf16, tag=f"OT{ec}", name=f"OT{ec}")
              for ec in range(ND)]
        for ci, s in enumerate(CST):
            for ec in range(ND):  # head pair (2*ec, 2*ec+1)
                # scores for the two heads go to the two BANKS of one psum
                # tile (two matmuls into one bank crash the exec unit, and
                # matmul M must be a multiple of 32)
                s2 = psS.tile([CL, 1024], f32, tag="psS", name="s2")
                nc.tensor.matmul(
                    s2[:, 0:P],
                    lhsT=QT[ec][0:64, s:s + CL],
                    rhs=KT[ec][0:64, s:s + P],
                    start=True, stop=True,
                )
                nc.tensor.matmul(
                    s2[:, 512:512 + P],
                    lhsT=QT[ec][64:128, s:s + CL],
                    rhs=KT[ec][64:128, s:s + P],
                    start=True, stop=True,
                )
                sv = s2.rearrange("p (b c) -> p b c", b=2)[:, :, 0:P]
                pm2 = work.tile([CL, 2 * P], f32, tag="pm2", name="pm2")
                nc.vector.scalar_tensor_tensor(
                    out=pm2.rearrange("p (b c) -> p b c", b=2),
                    in0=sv, scalar=1.0 / TEMP,
                    in1=msk_sb.rearrange("p (b c) -> p b c", b=2),
                    op0=Alu.mult, op1=Alu.add,
                )
                pe2 = work.tile([CL, 2 * P], f32, tag="pe2", name="pe2")
                nc.scalar.activation(out=pe2, in_=pm2, func=Act.Exp)
                rs2 = small.tile([CL, 2], f32, tag="rs2", name="rs2")
                nc.vector.tensor_reduce(
                    out=rs2,
                    in_=pe2.rearrange("a (h w) -> a h w", h=2),
                    axis=mybir.AxisListType.X, op=Alu.add,
                )
                rsr2 = small.tile([CL, 2], f32, tag="rsr2", name="rsr2")
                nc.vector.reciprocal(rsr2, rs2)
                pn2 = work.tile([CL, 2 * P], bf16, tag="pn2", name="pn2")
                nc.vector.tensor_tensor(
                    pn2.rearrange("a (h w) -> a h w", h=2),
                    pe2.rearrange("a (h w) -> a h w", h=2),
                    rsr2[:, :, None].to_broadcast((CL, 2, P)),
                    Alu.mult,
                )
                pt_ps = psT.tile([P, 2 * P], bf16, tag="psT", name="pt_ps")
                nc.tensor.transpose(pt_ps[:, 0:CL], pn2[:, 0:P],
                                    idn_sb[0:CL, 0:CL])
                nc.tensor.transpose(pt_ps[:, P:P + CL], pn2[:, P:2 * P],
                                    idn_sb[0:CL, 0:CL])
                pt_sb = work.tile([P, 2 * P], bf16, tag="ptsb", name="pt_sb")
                nc.vector.tensor_copy(pt_sb[:, 0:CL], pt_ps[:, 0:CL])
                nc.scalar.activation(out=pt_sb[:, P:P + CL],
                                     in_=pt_ps[:, P:P + CL], func=Act.Copy)
                ot2 = psO.tile([P, CL], f32, tag="psO", name="ot2")
                nc.tensor.matmul(
                    ot2[0:64, :],
                    lhsT=V[ci][:, ec * P:ec * P + 64],
                    rhs=pt_sb[:, 0:CL], start=True, stop=True,
                )
                nc.tensor.matmul(
                    ot2[64:128, :],
                    lhsT=V[ci][:, ec * P + 64:(ec + 1) * P],
                    rhs=pt_sb[:, P:P + CL], start=True, stop=True,
                )
                nc.scalar.activation(out=OT[ec][:, s:s + CL], in_=ot2,
                                     func=Act.Copy)

        # ---- FC + rank-1 bias + residual + layernorm ----
        for c in range(NT):
            cs = slice(c * P, (c + 1) * P)
            xr = xrpool.tile([P, D], f32, tag="xr", name="xr")
            nc.sync.dma_start(out=xr, in_=xq32[cs, :])
            y_sb = lnpool.tile([P, D], f32, tag="ysb", name="y_sb")
            psa = psA.tile([P, 512], f32, tag="psA", name="ps_fa")
            psb = psA.tile([P, 512], f32, tag="psA", name="ps_fb")
            for ec in range(ND):
                lt = OT[ec][:, cs]
                nc.tensor.matmul(psa, lhsT=lt, rhs=wf_t[ec][:, 0:512],
                                 start=(ec == 0), stop=False)
                nc.tensor.matmul(psb, lhsT=lt, rhs=wf_t[ec][:, 512:1024],
                                 start=(ec == 0), stop=False)
            nc.tensor.matmul(psa, lhsT=ones_sb, rhs=bpr_sb[:, 0:512],
                             start=False, stop=True)
            nc.tensor.matmul(psb, lhsT=ones_sb, rhs=bpr_sb[:, 512:1024],
                             start=False, stop=True)
            ysum = None
            for dcol, ps in ((0, psa), (1, psb)):
                ds_ = slice(dcol * 512, (dcol + 1) * 512)
                # NOTE: fused PSUM-source tensor_tensor_reduce crashes the
                # exec unit on HW; split copy + sbuf add + reduce.
                nc.vector.tensor_copy(y_sb[:, ds_], ps)
                nc.vector.tensor_add(y_sb[:, ds_], y_sb[:, ds_], xr[:, ds_])
                new_sum = small.tile([P, 1], f32, tag=f"ysum{dcol}",
                                     name="ysum")
                nc.vector.tensor_reduce(
                    out=new_sum, in_=y_sb[:, ds_],
                    axis=mybir.AxisListType.X, op=Alu.add,
                )
                if ysum is not None:
                    nsum2 = small.tile([P, 1], f32, tag="nsum2", name="nsum2")
                    nc.vector.tensor_add(nsum2, new_sum, ysum)
                    new_sum = nsum2
                ysum = new_sum
            sqs = []
            for dcol in range(2):
                ds_ = slice(dcol * 512, (dcol + 1) * 512)
                ysq = lnpool.tile([P, 512], f32, tag="ysq", name="ysq")
                sq = small.tile([P, 1], f32, tag=f"sq{dcol}", name="sq")
                nc.scalar.activation(out=ysq, in_=y_sb[:, ds_],
                                     func=Act.Square, accum_out=sq)
                sqs.append(sq)
            ssum = small.tile([P, 1], f32, tag="ssum", name="ssum")
            nc.vector.tensor_add(ssum, sqs[0], sqs[1])
            mean = small.tile([P, 1], f32, tag="mean", name="mean")
            nc.vector.tensor_scalar_mul(mean, ysum, 1.0 / D)
            msq = small.tile([P, 1], f32, tag="msq", name="msq")
            nc.vector.tensor_mul(msq, mean, mean)
            var = small.tile([P, 1], f32, tag="var", name="var")
            nc.vector.scalar_tensor_tensor(
                out=var, in0=ssum, scalar=1.0 / D, in1=msq,
                op0=Alu.mult, op1=Alu.subtract,
            )
            std = small.tile([P, 1], f32, tag="std", name="std")
            nc.scalar.activation(out=std, in_=var, func=Act.Sqrt, bias=eps_sb)
            rstd = small.tile([P, 1], f32, tag="rstd", name="rstd")
            nc.vector.reciprocal(rstd, std)
            bact = small.tile([P, 1], f32, tag="bact", name="bact")
            nc.vector.scalar_tensor_tensor(
                out=bact, in0=mean, scalar=-1.0, in1=rstd,
                op0=Alu.mult, op1=Alu.mult,
            )
            out_sb = lnpool.tile([P, D], f32, tag="osb", name="out_sb")
            nc.scalar.activation(out=out_sb, in_=y_sb, func=Act.Identity,
                                 bias=bact, scale=rstd)
            if apply_affine:
                nc.vector.tensor_mul(out_sb, out_sb, g_bc)
                nc.vector.tensor_add(out_sb, out_sb, b_bc)
            nc.sync.dma_start(out=yo[cs, :], in_=out_sb)

    nc.compile()
    return nc


def _get_program(apply_affine: bool):
    key = ("prog", apply_affine)
    if key not in _CACHE:
        _CACHE[key] = _build_program(apply_affine)
    return _CACHE[key]


def _host_prep(inputs):
    x = np.asarray(inputs["x"], np.float32)
    xs = np.asarray(inputs["xs"], np.float32)
    w_qs = np.asarray(inputs["w_qs"], np.float32)
    b_qs = np.asarray(inputs["b_qs"], np.float32)
    w_ks = np.asarray(inputs["w_ks"], np.float32)
    w_vs = np.asarray(inputs["w_vs"], np.float32)
    b_vs = np.asarray(inputs["b_vs"], np.float32)
    w_fc = np.asarray(inputs["w_fc"], np.float32)
    b_fc = np.asarray(inputs["b_fc"], np.float32)
    ln_g = np.asarray(inputs["ln_g"], np.float32)
    ln_b = np.asarray(inputs["ln_b"], np.float32)

    apply_affine = not (np.all(ln_g == 1.0) and np.all(ln_b == 0.0))

    bprime = (b_vs @ w_fc + b_fc).astype(np.float32)

    mask = np.full((CL, P), NEG, np.float32)
    for t in range(CL):
        mask[t, t:t + 2 * NEI + 1] = 0.0   # window cols beyond CL+6 stay NEG
    mask2 = np.concatenate([mask, mask], axis=1)

    shared = {
        "wq": np.ascontiguousarray(w_qs.astype(BF16)),
        "wk": np.ascontiguousarray(w_ks.astype(BF16)),
        "wv": np.ascontiguousarray(w_vs.astype(BF16)),
        "wf": np.ascontiguousarray(w_fc.astype(BF16)),
        "bq": np.ascontiguousarray(b_qs.reshape(ND, P).T.astype(np.float32)),
        "bpr": np.ascontiguousarray(bprime.reshape(1, D).astype(BF16)),
        "msk": np.ascontiguousarray(mask2),
        "idn": np.eye(P, dtype=BF16),
        "ones": np.ones((1, P), BF16),
    }
    if apply_affine:
        shared["lng"] = np.ascontiguousarray(ln_g.reshape(1, D))
        shared["lnb"] = np.ascontiguousarray(ln_b.reshape(1, D))

    in_maps = []
    half_n = S // 2  # 1024
    for core in range(NCORES):
        b, half = core // 2, core % 2
        t0 = half * half_n
        xq = x[b, t0:t0 + half_n]
        halo = np.zeros((TH, D), np.float32)
        lo = max(0, t0 - NEI)
        hi = min(S, t0 + half_n + NEI)
        halo[lo - (t0 - NEI):hi - (t0 - NEI)] = xs[b, lo:hi]
        m = dict(shared)
        m["xq32"] = np.ascontiguousarray(xq)
        m["xqT"] = np.ascontiguousarray(xq.T.astype(BF16))
        m["xsT"] = np.ascontiguousarray(halo.T.astype(BF16))
        in_maps.append(m)
    return in_maps, apply_affine


def _run(inputs, trace=False, trace_kwargs=None):
    from concourse.bass_utils import run_bass_kernel_spmd

    in_maps, apply_affine = _host_prep(inputs)
    nc = _get_program(apply_affine)
    res = run_bass_kernel_spmd(
        nc, in_maps, list(range(NCORES)),
        trace=trace, **(trace_kwargs or {})
    )
    y = np.empty((B, S, D), np.float32)
    half_n = S // 2
    for core in range(NCORES):
        b, half = core // 2, core % 2
        y[b, half * half_n:(half + 1) * half_n] = res.results[core]["yo"]
    return y, res


def kernel(**inputs):
    y, _ = _run(inputs)
    return y



# revision 6
# speedup vs baseline: 1.2107x; 1.2107x over previous
"""LocalSelfAttention (window=7) Trainium2 Bass kernel, v2.

Full inputs in, full output out. Sharding: 8 cores = batch(4) x seq-half(2),
each core handles 1024 tokens with a 3-token zero-padded halo on xs.

Math rewrites (all exact or bf16-rounding-only):
- K bias drops (softmax shift invariance, incl. zero-padded halo taps).
- V bias + FC bias fold into the residual on the host:
  xq' = x + (b_vs @ w_fc + b_fc); residual enters the FC psum via an
  identity matmul so no DVE copy/add is needed.
- 1/sqrt(dk) folds into the Q projection eviction (ACT scale).
- Band masking is a 0/1 multiply AFTER exp (garbage scores are bounded,
  exp stays finite), so scores go straight from PSUM through one wide
  ACT exp into bf16 SBUF per 8-head group.

Attention: 11 chunks of 96 queries, 102-wide key windows. Scores for 8
heads share one 2-bank PSUM tile (head slots at 128-f32 stride). Softmax
is 3 wide DVE ops per group (band-mult, reduce, normalize-mult) plus a
tiny reciprocal. Per-head PE transposes (identity matmul) and single
PV matmuls (V is projected as 11 window-aligned chunk tiles).

PSUM budget (8 banks): scores 2 + transpose-out 2 + PV-out 2 + proj/FC 2.
"""

import sys

for _p in ("/opt/trn_rl_repo",):
    if _p not in sys.path:
        sys.path.insert(0, _p)

import numpy as np
import ml_dtypes

BF16 = ml_dtypes.bfloat16

H, DK, DV, D = 16, 64, 64, 1024
NEI = 3
TEMP = 8.0
EPS = 1e-5
B, S = 4, 2048
NCORES = 8
T = (B * S) // NCORES          # 1024 tokens per core
TH = T + 2 * NEI               # 1030 halo tokens
P = 128
NT = T // P                    # 8 fc-phase token chunks
ND = D // P                    # 8 feature chunks
CL = 96                        # attention chunk length
NCH = 11                       # attention chunks
TQ = 1056                      # padded query width (NCH*CL)
KW = 1088                      # padded key block width (windows read 128)
XSW = 1152                     # padded xsT block width (V window reads)
W = 102                        # key window (CL + 2*NEI); last chunk: 96

_CACHE = {}


def _build_program(apply_affine: bool):
    import concourse.bacc as bacc
    import concourse.tile as tile
    import concourse.bass as bass
    from concourse import mybir
    from contextlib import ExitStack

    f32 = mybir.dt.float32
    bf16 = mybir.dt.bfloat16
    Alu = mybir.AluOpType
    Act = mybir.ActivationFunctionType

    nc = bacc.Bacc(
        "TRN2", target_bir_lowering=False, debug=False, enable_asserts=False
    )

    def din(name, shape, dt_):
        return nc.dram_tensor(name, shape, dt_, kind="ExternalInput").ap()

    xqT = din("xqT", (D, T), bf16)        # x^T (host-transposed)
    xsT = din("xsT", (D, TH), bf16)       # xs^T halo (host-transposed)
    xq = din("xq", (T, D), bf16)          # x + bprime, token-major
    wq = din("wq", (D, D), bf16)
    wk = din("wk", (D, D), bf16)
    wv = din("wv", (D, D), bf16)
    wf = din("wf", (D, D), bf16)
    bq = din("bq", (P, ND), f32)          # b_qs/TEMP laid out [p, ec]
    band = din("band", (CL, P), bf16)     # 0/1 band mask (cols>=102 zero)
    idn = din("idn", (P, P), bf16)        # identity for PE transpose/residual
    if apply_affine:
        lng = din("lng", (1, D), f32)
        lnb = din("lnb", (1, D), f32)
    yo = nc.dram_tensor("yo", (T, D), f32, kind="ExternalOutput").ap()

    def dram_blocks_ap(src, nblk, blk_w, valid_w=None):
        """AP over src (R, C) viewed as [128 par, nblk, valid_w] where
        block b, partition p reads src row 128*b + p, cols 0:valid_w."""
        vw = valid_w if valid_w is not None else blk_w
        c = src.ap[-1][0]  # innermost stride (elements)
        rstride = src.ap[0][0]
        return bass.AP(
            tensor=src.tensor, offset=src.offset,
            ap=[[rstride, P], [rstride * P, nblk], [c, vw]],
        )

    with tile.TileContext(nc) as tc, ExitStack() as ctx:
        consts = ctx.enter_context(tc.tile_pool(name="consts", bufs=1))
        big = ctx.enter_context(tc.tile_pool(name="big", bufs=1))
        wpool = ctx.enter_context(tc.tile_pool(name="wpool", bufs=2))
        pepool = ctx.enter_context(tc.tile_pool(name="pepool", bufs=2))
        pnpool = ctx.enter_context(tc.tile_pool(name="pnpool", bufs=2))
        ptpool = ctx.enter_context(tc.tile_pool(name="ptpool", bufs=2))
        ypool = ctx.enter_context(tc.tile_pool(name="ypool", bufs=2))
        small = ctx.enter_context(tc.tile_pool(name="small", bufs=3))
        lns = ctx.enter_context(tc.tile_pool(name="lns", bufs=2))
        psA = ctx.enter_context(tc.tile_pool(name="psA", bufs=2, space="PSUM"))
        psS = ctx.enter_context(tc.tile_pool(name="psS", bufs=1, space="PSUM"))
        psT = ctx.enter_context(tc.tile_pool(name="psT", bufs=2, space="PSUM"))
        psO = ctx.enter_context(tc.tile_pool(name="psO", bufs=2, space="PSUM"))

        # ---- constants ----
        idn_sb = consts.tile([P, P], bf16, tag="idn")
        nc.sync.dma_start(out=idn_sb, in_=idn)
        band_sb = consts.tile([CL, P], bf16, tag="band")
        nc.sync.dma_start(out=band_sb, in_=band)
        bq_sb = consts.tile([P, ND], f32, tag="bq")
        nc.sync.dma_start(out=bq_sb, in_=bq)
        eps_sb = consts.tile([P, 1], f32, tag="eps")
        nc.vector.memset(eps_sb, EPS)
        if apply_affine:
            g_bc = consts.tile([P, D], f32, tag="g_bc")
            b_bc = consts.tile([P, D], f32, tag="b_bc")
            nc.sync.dma_start(
                out=g_bc,
                in_=bass.AP(tensor=lng.tensor, offset=lng.offset,
                            ap=[[0, P]] + list(lng.ap[1:])),
            )
            nc.sync.dma_start(
                out=b_bc,
                in_=bass.AP(tensor=lnb.tensor, offset=lnb.offset,
                            ap=[[0, P]] + list(lnb.ap[1:])),
            )

        # ---- big activation tiles (single DMAs) ----
        xT_all = big.tile([P, ND * T], bf16, tag="xT")
        nc.sync.dma_start(
            out=xT_all.rearrange("p (b c) -> p b c", b=ND),
            in_=dram_blocks_ap(xqT, ND, T),
        )
        wq_sb = wpool.tile([P, ND * D], bf16, tag="w", name="wq")
        nc.sync.dma_start(
            out=wq_sb.rearrange("p (b c) -> p b c", b=ND),
            in_=dram_blocks_ap(wq, ND, D),
        )
        xsT_all = big.tile([P, ND * XSW], bf16, tag="xsT")
        xsv = xsT_all.rearrange("p (b c) -> p b c", b=ND)
        nc.sync.dma_start(out=xsv[:, :, 0:TH], in_=dram_blocks_ap(xsT, ND, XSW, TH))
        nc.vector.memset(xsv[:, :, TH:XSW], 0.0)
        wk_sb = wpool.tile([P, ND * D], bf16, tag="w", name="wk")
        nc.sync.dma_start(
            out=wk_sb.rearrange("p (b c) -> p b c", b=ND),
            in_=dram_blocks_ap(wk, ND, D),
        )

        QT = big.tile([P, ND * TQ], bf16, tag="QT")
        KT = big.tile([P, ND * KW], bf16, tag="KT")
        V_all = big.tile([P, NCH * D], bf16, tag="V")
        OT = big.tile([P, ND * T], bf16, tag="OT")

        # ---- Q projection: feature-major, bias+1/TEMP via ACT evict ----
        for ec in range(ND):
            psa = psA.tile([P, 512], f32, tag="psA", name="ps_qa")
            psb = psA.tile([P, 512], f32, tag="psA", name="ps_qb")
            for dc in range(ND):
                nc.tensor.matmul(psa, lhsT=wq_sb[:, dc * D + ec * P:dc * D + ec * P + P],
                                 rhs=xT_all[:, dc * T:dc * T + 512],
                                 start=(dc == 0), stop=(dc == ND - 1))
            for dc in range(ND):
                nc.tensor.matmul(psb, lhsT=wq_sb[:, dc * D + ec * P:dc * D + ec * P + P],
                                 rhs=xT_all[:, dc * T + 512:dc * T + 1024],
                                 start=(dc == 0), stop=(dc == ND - 1))
            nc.scalar.activation(out=QT[:, ec * TQ:ec * TQ + 512], in_=psa,
                                 func=Act.Identity,
                                 bias=bq_sb[:, ec:ec + 1], scale=1.0 / TEMP)
            nc.vector.scalar_tensor_tensor(
                out=QT[:, ec * TQ + 512:ec * TQ + 1024], in0=psb,
                scalar=1.0 / TEMP,
                in1=bq_sb[:, ec:ec + 1].to_broadcast((P, 512)),
                op0=Alu.mult, op1=Alu.add,
            )
        # zero the 32 pad query columns of each block
        qv = QT.rearrange("p (b c) -> p b c", b=ND)
        nc.vector.memset(qv[:, :, T:TQ], 0.0)

        wv_sb = wpool.tile([P, ND * D], bf16, tag="w", name="wv")
        nc.sync.dma_start(
            out=wv_sb.rearrange("p (b c) -> p b c", b=ND),
            in_=dram_blocks_ap(wv, ND, D),
        )

        # ---- K projection (no bias); KT cols beyond 1030 are 0 ----
        for ec in range(ND):
            psa = psA.tile([P, 512], f32, tag="psA", name="ps_ka")
            psb = psA.tile([P, 512], f32, tag="psA", name="ps_kb")
            for dc in range(ND):
                nc.tensor.matmul(psa, lhsT=wk_sb[:, dc * D + ec * P:dc * D + ec * P + P],
                                 rhs=xsT_all[:, dc * XSW:dc * XSW + 512],
                                 start=(dc == 0), stop=(dc == ND - 1))
            for dc in range(ND):
                nc.tensor.matmul(psb, lhsT=wk_sb[:, dc * D + ec * P:dc * D + ec * P + P],
                                 rhs=xsT_all[:, dc * XSW + 512:dc * XSW + 1024],
                                 start=(dc == 0), stop=(dc == ND - 1))
            pst = psA.tile([P, KW - 1024], f32, tag="psA", name="ps_kt")
            for dc in range(ND):
                nc.tensor.matmul(pst,
                                 lhsT=wk_sb[:, dc * D + ec * P:dc * D + ec * P + P],
                                 rhs=xsT_all[:, dc * XSW + 1024:dc * XSW + KW],
                                 start=(dc == 0), stop=(dc == ND - 1))
            nc.scalar.activation(out=KT[:, ec * KW:ec * KW + 512], in_=psa,
                                 func=Act.Copy)
            nc.vector.tensor_copy(KT[:, ec * KW + 512:ec * KW + 1024], psb)
            nc.scalar.activation(out=KT[:, ec * KW + 1024:ec * KW + KW], in_=pst,
                                 func=Act.Copy)

        wf_sb = wpool.tile([P, ND * D], bf16, tag="w", name="wf")
        nc.sync.dma_start(
            out=wf_sb.rearrange("p (b c) -> p b c", b=ND),
            in_=dram_blocks_ap(wf, ND, D),
        )
        xq_all = big.tile([P, NT * D], bf16, tag="xq")
        nc.sync.dma_start(
            out=xq_all.rearrange("p (b c) -> p b c", b=NT),
            in_=dram_blocks_ap(xq, NT, D),
        )

        # ---- V projection: 11 window-aligned chunk tiles (halo rows) ----
        for ci in range(NCH):
            s = CL * ci
            psa = psA.tile([P, 512], f32, tag="psA", name="ps_va")
            psb = psA.tile([P, 512], f32, tag="psA", name="ps_vb")
            for dc in range(ND):
                nc.tensor.matmul(psa, lhsT=xsT_all[:, dc * XSW + s:dc * XSW + s + P],
                                 rhs=wv_sb[:, dc * D:dc * D + 512],
                                 start=(dc == 0), stop=(dc == ND - 1))
            for dc in range(ND):
                nc.tensor.matmul(psb, lhsT=xsT_all[:, dc * XSW + s:dc * XSW + s + P],
                                 rhs=wv_sb[:, dc * D + 512:dc * D + 1024],
                                 start=(dc == 0), stop=(dc == ND - 1))
            nc.scalar.activation(out=V_all[:, ci * D:ci * D + 512], in_=psa,
                                 func=Act.Copy)
            nc.vector.tensor_copy(V_all[:, ci * D + 512:ci * D + 1024], psb)

        # ---- attention + FC, software-pipelined ----
        # FC chunk c is emitted once PV of its source chunks is emitted.
        fc_at = {2: [0], 3: [1], 4: [2], 6: [3], 7: [4], 8: [5], 10: [6], 12: [7]}

        # slot sl of a group holds head 8g + perm(sl); slots 0-3 (psum bank 0)
        # take the partition-base-0 heads, slots 4-7 (bank 1) the base-64
        # heads: consecutive matmuls into one PSUM bank must share the PE
        # tile row (lhsT partition base) or the exec unit dies.
        def s_head(g, sl):
            return 8 * g + (sl % 4) * 2 + sl // 4

        def emit_scores(ci, g):
            s = CL * ci
            s2 = psS.tile([CL, 1024], f32, tag="psS", name=f"s2_{ci}_{g}")
            for sl in range(8):
                ec = 4 * g + sl % 4
                r = sl // 4
                nc.tensor.matmul(
                    s2[:, sl * P:sl * P + P],
                    lhsT=QT[64 * r:64 * r + 64, ec * TQ + s:ec * TQ + s + CL],
                    rhs=KT[64 * r:64 * r + 64, ec * KW + s:ec * KW + s + P],
                    start=True, stop=True,
                )
            return s2

        def emit_softmax(ci, g, s2):
            pe = pepool.tile([CL, 1024], bf16, tag="pe", name=f"pe_{ci}_{g}")
            nc.scalar.activation(out=pe, in_=s2, func=Act.Exp)
            pev = pe.rearrange("p (h c) -> p h c", h=8)
            nc.vector.tensor_tensor(
                pev, pev,
                band_sb[:, None, :].to_broadcast((CL, 8, P)),
                Alu.mult,
            )
            den = small.tile([CL, 8], f32, tag=f"den{g}", name="den")
            nc.vector.tensor_reduce(out=den, in_=pev,
                                    axis=mybir.AxisListType.X, op=Alu.add)
            rcp = small.tile([CL, 8], f32, tag=f"rcp{g}", name="rcp")
            nc.vector.reciprocal(rcp, den)
            pn = pnpool.tile([CL, 1024], bf16, tag="pn", name=f"pn_{ci}_{g}")
            nc.vector.tensor_tensor(
                pn.rearrange("p (h c) -> p h c", h=8),
                pev,
                rcp[:, :, None].to_broadcast((CL, 8, P)),
                Alu.mult,
            )
            return pn

        def emit_transposes(ci, g, pn):
            pt = psT.tile([P, 1024], bf16, tag="psT", name=f"pt_{ci}_{g}")
            for h in range(8):
                nc.tensor.transpose(pt[:, h * P:h * P + CL],
                                    pn[:, h * P:h * P + P],
                                    idn_sb[0:CL, 0:CL])
            ptsb = ptpool.tile([P, 1024], bf16, tag="pt", name=f"ptsb_{ci}_{g}")
            src = pt.rearrange("p (h c) -> p h c", h=8)[:, :, 0:CL]
            dst = ptsb.rearrange("p (h c) -> p h c", h=8)[:, :, 0:CL]
            if g == 0:
                nc.vector.tensor_copy(dst, src)
            else:
                nc.scalar.activation(out=dst, in_=src, func=Act.Copy)
            return ptsb

        def emit_pv(ci, g, ptsb):
            ot = psO.tile([P, 512], f32, tag="psO", name=f"ot_{ci}_{g}")
            for sl in range(8):
                hh = s_head(g, sl)
                hl = hh - 8 * g
                p_, r = hl // 2, hl % 2
                nc.tensor.matmul(
                    ot[64 * r:64 * r + 64, p_ * P:p_ * P + CL],
                    lhsT=V_all[:, ci * D + hh * DV:ci * D + hh * DV + DV],
                    rhs=ptsb[:, sl * P:sl * P + CL],
                    start=True, stop=True,
                )
            # evict: pair p of this group -> OT block (4g + p), token cols
            s = CL * ci
            ew = CL if ci < NCH - 1 else T - s   # last chunk: only 64 valid
            otv = OT.rearrange("p (b c) -> p b c", b=ND)
            nc.scalar.activation(
                out=otv[:, 4 * g:4 * g + 4, s:s + ew],
                in_=ot.rearrange("p (h c) -> p h c", h=4)[:, :, 0:ew],
                func=Act.Copy,
            )

        def emit_fc(c):
            cs = c * P
            psa = psA.tile([P, 512], f32, tag="psA", name=f"ps_fa{c}")
            psb = psA.tile([P, 512], f32, tag="psA", name=f"ps_fb{c}")
            for ec in range(ND):
                nc.tensor.matmul(psa, lhsT=OT[:, ec * T + cs:ec * T + cs + P],
                                 rhs=wf_sb[:, ec * D:ec * D + 512],
                                 start=(ec == 0), stop=False)
            nc.tensor.matmul(psa, lhsT=idn_sb,
                             rhs=xq_all[:, c * D:c * D + 512],
                             start=False, stop=True)
            for ec in range(ND):
                nc.tensor.matmul(psb, lhsT=OT[:, ec * T + cs:ec * T + cs + P],
                                 rhs=wf_sb[:, ec * D + 512:ec * D + 1024],
                                 start=(ec == 0), stop=False)
            nc.tensor.matmul(psb, lhsT=idn_sb,
                             rhs=xq_all[:, c * D + 512:c * D + 1024],
                             start=False, stop=True)
            # layernorm on psum y = fc + residual
            s0 = lns.tile([P, 1], f32, tag="s0", name="s0")
            s1 = lns.tile([P, 1], f32, tag="s1", name="s1")
            nc.vector.tensor_reduce(out=s0, in_=psa,
                                    axis=mybir.AxisListType.X, op=Alu.add)
            nc.vector.tensor_reduce(out=s1, in_=psb,
                                    axis=mybir.AxisListType.X, op=Alu.add)
            q0 = lns.tile([P, 1], f32, tag="q0", name="q0")
            q1 = lns.tile([P, 1], f32, tag="q1", name="q1")
            ysq = lns.tile([P, 512], f32, tag="ysq", name="ysq")
            nc.scalar.activation(out=ysq, in_=psa, func=Act.Square, accum_out=q0)
            nc.scalar.activation(out=ysq, in_=psb, func=Act.Square, accum_out=q1)
            ysum = lns.tile([P, 1], f32, tag="ysum", name="ysum")
            nc.vector.tensor_add(ysum, s0, s1)
            ssum = lns.tile([P, 1], f32, tag="ssum", name="ssum")
            nc.vector.tensor_add(ssum, q0, q1)
            mean = lns.tile([P, 1], f32, tag="mean", name="mean")
            nc.vector.tensor_scalar_mul(mean, ysum, 1.0 / D)
            msq = lns.tile([P, 1], f32, tag="msq", name="msq")
            nc.vector.tensor_mul(msq, mean, mean)
            var = lns.tile([P, 1], f32, tag="var", name="var")
            nc.vector.scalar_tensor_tensor(
                out=var, in0=ssum, scalar=1.0 / D, in1=msq,
                op0=Alu.mult, op1=Alu.subtract,
            )
            std = lns.tile([P, 1], f32, tag="std", name="std")
            nc.scalar.activation(out=std, in_=var, func=Act.Sqrt, bias=eps_sb)
            rstd = lns.tile([P, 1], f32, tag="rstd", name="rstd")
            nc.vector.reciprocal(rstd, std)
            bact = lns.tile([P, 1], f32, tag="bact", name="bact")
            nc.vector.scalar_tensor_tensor(
                out=bact, in0=mean, scalar=-1.0, in1=rstd,
                op0=Alu.mult, op1=Alu.mult,
            )
            y = ypool.tile([P, D], f32, tag="y", name=f"y{c}")
            nc.scalar.activation(out=y[:, 0:512], in_=psa, func=Act.Identity,
                                 bias=bact, scale=rstd)
            nc.scalar.activation(out=y[:, 512:1024], in_=psb, func=Act.Identity,
                                 bias=bact, scale=rstd)
            if apply_affine:
                nc.vector.tensor_mul(y, y, g_bc)
                nc.vector.tensor_add(y, y, b_bc)
            nc.sync.dma_start(out=yo[cs:cs + P, :], in_=y)

        pn_prev = [None, None]
        for ci in range(13):
            if ci < NCH:
                s2a = emit_scores(ci, 0)
            if 1 <= ci <= NCH:
                pta = emit_transposes(ci - 1, 0, pn_prev[0])
            if ci < NCH:
                s2b = emit_scores(ci, 1)
            if 1 <= ci <= NCH:
                ptb = emit_transposes(ci - 1, 1, pn_prev[1])
                emit_pv(ci - 1, 0, pta)
                emit_pv(ci - 1, 1, ptb)
            if ci < NCH:
                pn_prev[0] = emit_softmax(ci, 0, s2a)
                pn_prev[1] = emit_softmax(ci, 1, s2b)
            for c in fc_at.get(ci, []):
                emit_fc(c)

    nc.compile()
    return nc


def _get_program(apply_affine: bool):
    key = ("prog", apply_affine)
    if key not in _CACHE:
        _CACHE[key] = _build_program(apply_affine)
    return _CACHE[key]


def _host_prep(inputs):
    x = np.asarray(inputs["x"], np.float32)
    xs = np.asarray(inputs["xs"], np.float32)
    w_qs = np.asarray(inputs["w_qs"], np.float32)
    b_qs = np.asarray(inputs["b_qs"], np.float32)
    w_ks = np.asarray(inputs["w_ks"], np.float32)
    w_vs = np.asarray(inputs["w_vs"], np.float32)
    b_vs = np.asarray(inputs["b_vs"], np.float32)
    w_fc = np.asarray(inputs["w_fc"], np.float32)
    b_fc = np.asarray(inputs["b_fc"], np.float32)
    ln_g = np.asarray(inputs["ln_g"], np.float32)
    ln_b = np.asarray(inputs["ln_b"], np.float32)

    apply_affine = not (np.all(ln_g == 1.0) and np.all(ln_b == 0.0))

    bprime = (b_vs @ w_fc + b_fc).astype(np.float32)

    band = np.zeros((CL, P), np.float32)
    for t in range(CL):
        band[t, t:t + 2 * NEI + 1] = 1.0

    shared = {
        "wq": np.ascontiguousarray(w_qs.astype(BF16)),
        "wk": np.ascontiguousarray(w_ks.astype(BF16)),
        "wv": np.ascontiguousarray(w_vs.astype(BF16)),
        "wf": np.ascontiguousarray(w_fc.astype(BF16)),
        "bq": np.ascontiguousarray(
            (b_qs / TEMP).reshape(ND, P).T.astype(np.float32)),
        "band": np.ascontiguousarray(band.astype(BF16)),
        "idn": np.eye(P, dtype=BF16),
    }
    if apply_affine:
        shared["lng"] = np.ascontiguousarray(ln_g.reshape(1, D))
        shared["lnb"] = np.ascontiguousarray(ln_b.reshape(1, D))

    in_maps = []
    half_n = S // 2  # 1024
    for core in range(NCORES):
        b, half = core // 2, core % 2
        t0 = half * half_n
        xc = x[b, t0:t0 + half_n]
        halo = np.zeros((TH, D), np.float32)
        lo = max(0, t0 - NEI)
        hi = min(S, t0 + half_n + NEI)
        halo[lo - (t0 - NEI):hi - (t0 - NEI)] = xs[b, lo:hi]
        m = dict(shared)
        m["xq"] = np.ascontiguousarray((xc + bprime).astype(BF16))
        m["xqT"] = np.ascontiguousarray(xc.T.astype(BF16))
        m["xsT"] = np.ascontiguousarray(halo.T.astype(BF16))
        in_maps.append(m)
    return in_maps, apply_affine


def _run(inputs, trace=False, trace_kwargs=None):
    from concourse.bass_utils import run_bass_kernel_spmd

    in_maps, apply_affine = _host_prep(inputs)
    nc = _get_program(apply_affine)
    res = run_bass_kernel_spmd(
        nc, in_maps, list(range(NCORES)),
        trace=trace, **(trace_kwargs or {})
    )
    y = np.empty((B, S, D), np.float32)
    half_n = S // 2
    for core in range(NCORES):
        b, half = core // 2, core % 2
        y[b, half * half_n:(half + 1) * half_n] = res.results[core]["yo"]
    return y, res


def kernel(**inputs):
    y, _ = _run(inputs)
    return y


# revision 8
# speedup vs baseline: 1.2583x; 1.0394x over previous
"""LocalSelfAttention (window=7) Trainium2 Bass kernel, v2.

Full inputs in, full output out. Sharding: 8 cores = batch(4) x seq-half(2),
each core handles 1024 tokens with a 3-token zero-padded halo on xs.

Math rewrites (all exact or bf16-rounding-only):
- K bias drops (softmax shift invariance, incl. zero-padded halo taps).
- V bias + FC bias fold into the residual on the host:
  xq' = x + (b_vs @ w_fc + b_fc); residual enters the FC psum via an
  identity matmul so no DVE copy/add is needed.
- 1/sqrt(dk) folds into the Q projection eviction (ACT scale).
- Band masking is a 0/1 multiply AFTER exp (garbage scores are bounded,
  exp stays finite), so scores go straight from PSUM through one wide
  ACT exp into bf16 SBUF per 8-head group.

Attention: 11 chunks of 96 queries, 102-wide key windows. Scores for 8
heads share one 2-bank PSUM tile (head slots at 128-f32 stride). Softmax
is 3 wide DVE ops per group (band-mult, reduce, normalize-mult) plus a
tiny reciprocal. Per-head PE transposes (identity matmul) and single
PV matmuls (V is projected as 11 window-aligned chunk tiles).

PSUM budget (8 banks): scores 2 + transpose-out 2 + PV-out 2 + proj/FC 2.
"""

import sys

for _p in ("/opt/trn_rl_repo",):
    if _p not in sys.path:
        sys.path.insert(0, _p)

import numpy as np
import ml_dtypes

BF16 = ml_dtypes.bfloat16

H, DK, DV, D = 16, 64, 64, 1024
NEI = 3
TEMP = 8.0
EPS = 1e-5
B, S = 4, 2048
NCORES = 8
T = (B * S) // NCORES          # 1024 tokens per core
TH = T + 2 * NEI               # 1030 halo tokens
P = 128
NT = T // P                    # 8 fc-phase token chunks
ND = D // P                    # 8 feature chunks
CL = 96                        # attention chunk length
NCH = 11                       # attention chunks
TQ = 1056                      # padded query width (NCH*CL)
KW = 1088                      # padded key block width (windows read 128)
XSW = 1152                     # padded xsT block width (V window reads)
W = 102                        # key window (CL + 2*NEI); last chunk: 96

_CACHE = {}


def _build_program(apply_affine: bool):
    import concourse.bacc as bacc
    import concourse.tile as tile
    import concourse.bass as bass
    from concourse import mybir
    from contextlib import ExitStack

    f32 = mybir.dt.float32
    bf16 = mybir.dt.bfloat16
    Alu = mybir.AluOpType
    Act = mybir.ActivationFunctionType

    nc = bacc.Bacc(
        "TRN2", target_bir_lowering=False, debug=False, enable_asserts=False
    )

    def din(name, shape, dt_):
        return nc.dram_tensor(name, shape, dt_, kind="ExternalInput").ap()

    xqT = din("xqT", (D, T), bf16)        # x^T (host-transposed)
    xsT = din("xsT", (D, TH), bf16)       # xs^T halo (host-transposed)
    xq = din("xq", (T, D), bf16)          # x + bprime, token-major
    wq = din("wq", (D, D), bf16)
    wk = din("wk", (D, D), bf16)
    wv = din("wv", (D, D), bf16)
    wf = din("wf", (D, D), bf16)
    bq = din("bq", (P, ND), f32)          # b_qs/TEMP laid out [p, ec]
    band = din("band", (CL, P), bf16)     # 0/1 band mask (cols>=102 zero)
    idn = din("idn", (P, P), bf16)        # identity for PE transpose/residual
    if apply_affine:
        lng = din("lng", (1, D), f32)
        lnb = din("lnb", (1, D), f32)
    yo = nc.dram_tensor("yo", (T, D), f32, kind="ExternalOutput").ap()

    def dram_blocks_ap(src, nblk, blk_w, valid_w=None, b0=0):
        """AP over src (R, C) viewed as [128 par, nblk, valid_w] where
        block b0+b, partition p reads src row 128*(b0+b) + p, cols 0:valid_w."""
        vw = valid_w if valid_w is not None else blk_w
        c = src.ap[-1][0]  # innermost stride (elements)
        rstride = src.ap[0][0]
        return bass.AP(
            tensor=src.tensor, offset=src.offset + rstride * P * b0,
            ap=[[rstride, P], [rstride * P, nblk], [c, vw]],
        )

    with tile.TileContext(nc) as tc, ExitStack() as ctx:
        consts = ctx.enter_context(tc.tile_pool(name="consts", bufs=1))
        big = ctx.enter_context(tc.tile_pool(name="big", bufs=1))
        wpool = ctx.enter_context(tc.tile_pool(name="wpool", bufs=2))
        pepool = ctx.enter_context(tc.tile_pool(name="pepool", bufs=2))
        pnpool = ctx.enter_context(tc.tile_pool(name="pnpool", bufs=2))
        ptpool = ctx.enter_context(tc.tile_pool(name="ptpool", bufs=2))
        ypool = ctx.enter_context(tc.tile_pool(name="ypool", bufs=2))
        small = ctx.enter_context(tc.tile_pool(name="small", bufs=3))
        lns = ctx.enter_context(tc.tile_pool(name="lns", bufs=2))
        psA = ctx.enter_context(tc.tile_pool(name="psA", bufs=3, space="PSUM"))
        psS = ctx.enter_context(tc.tile_pool(name="psS", bufs=1, space="PSUM"))
        psT = ctx.enter_context(tc.tile_pool(name="psT", bufs=1, space="PSUM"))
        psO = ctx.enter_context(tc.tile_pool(name="psO", bufs=2, space="PSUM"))

        # ---- constants (vector queue; sync starts the big loads) ----
        idn_sb = consts.tile([P, P], bf16, tag="idn")
        nc.gpsimd.dma_start(out=idn_sb, in_=idn)
        band_sb = consts.tile([CL, P], bf16, tag="band")
        nc.gpsimd.dma_start(out=band_sb, in_=band)
        bq_sb = consts.tile([P, ND], f32, tag="bq")
        nc.gpsimd.dma_start(out=bq_sb, in_=bq)
        eps_sb = consts.tile([P, 1], f32, tag="eps")
        nc.vector.memset(eps_sb, EPS)
        if apply_affine:
            g_bc = consts.tile([P, D], f32, tag="g_bc")
            b_bc = consts.tile([P, D], f32, tag="b_bc")
            nc.sync.dma_start(
                out=g_bc,
                in_=bass.AP(tensor=lng.tensor, offset=lng.offset,
                            ap=[[0, P]] + list(lng.ap[1:])),
            )
            nc.sync.dma_start(
                out=b_bc,
                in_=bass.AP(tensor=lnb.tensor, offset=lnb.offset,
                            ap=[[0, P]] + list(lnb.ap[1:])),
            )

        # ---- big activation tiles: halves spread over 4 DMA queues ----
        xT_all = big.tile([P, ND * T], bf16, tag="xT")
        xtv = xT_all.rearrange("p (b c) -> p b c", b=ND)
        nc.sync.dma_start(out=xtv[:, 0:4, :], in_=dram_blocks_ap(xqT, 4, T))
        nc.scalar.dma_start(out=xtv[:, 4:8, :], in_=dram_blocks_ap(xqT, 4, T, b0=4))
        wq_sb = wpool.tile([P, ND * D], bf16, tag="w", name="wq")
        wqv = wq_sb.rearrange("p (b c) -> p b c", b=ND)
        nc.gpsimd.dma_start(out=wqv[:, 0:4, :], in_=dram_blocks_ap(wq, 4, D))
        nc.gpsimd.dma_start(out=wqv[:, 4:8, :], in_=dram_blocks_ap(wq, 4, D, b0=4))
        xsT_all = big.tile([P, ND * XSW], bf16, tag="xsT")
        xsv = xsT_all.rearrange("p (b c) -> p b c", b=ND)
        nc.sync.dma_start(out=xsv[:, 0:4, 0:TH],
                          in_=dram_blocks_ap(xsT, 4, XSW, TH))
        nc.scalar.dma_start(out=xsv[:, 4:8, 0:TH],
                            in_=dram_blocks_ap(xsT, 4, XSW, TH, b0=4))
        nc.vector.memset(xsv[:, :, TH:XSW], 0.0)
        wk_sb = wpool.tile([P, ND * D], bf16, tag="w", name="wk")
        wkv = wk_sb.rearrange("p (b c) -> p b c", b=ND)
        nc.gpsimd.dma_start(out=wkv[:, 0:4, :], in_=dram_blocks_ap(wk, 4, D))
        nc.gpsimd.dma_start(out=wkv[:, 4:8, :], in_=dram_blocks_ap(wk, 4, D, b0=4))

        QT = big.tile([P, ND * TQ], bf16, tag="QT")
        KT = big.tile([P, ND * KW], bf16, tag="KT")
        V_all = big.tile([P, NCH * D], bf16, tag="V")
        OT = big.tile([P, ND * T], bf16, tag="OT")

        # ---- Q projection: feature-major, bias+1/TEMP via ACT evict ----
        for ec in range(ND):
            psa = psA.tile([P, 512], f32, tag="psA", name="ps_qa")
            psb = psA.tile([P, 512], f32, tag="psA", name="ps_qb")
            for dc in range(ND):
                nc.tensor.matmul(psa, lhsT=wq_sb[:, dc * D + ec * P:dc * D + ec * P + P],
                                 rhs=xT_all[:, dc * T:dc * T + 512],
                                 start=(dc == 0), stop=(dc == ND - 1))
            for dc in range(ND):
                nc.tensor.matmul(psb, lhsT=wq_sb[:, dc * D + ec * P:dc * D + ec * P + P],
                                 rhs=xT_all[:, dc * T + 512:dc * T + 1024],
                                 start=(dc == 0), stop=(dc == ND - 1))
            nc.scalar.activation(out=QT[:, ec * TQ:ec * TQ + 512], in_=psa,
                                 func=Act.Identity,
                                 bias=bq_sb[:, ec:ec + 1], scale=1.0 / TEMP)
            nc.vector.scalar_tensor_tensor(
                out=QT[:, ec * TQ + 512:ec * TQ + 1024], in0=psb,
                scalar=1.0 / TEMP,
                in1=bq_sb[:, ec:ec + 1].to_broadcast((P, 512)),
                op0=Alu.mult, op1=Alu.add,
            )
        # zero the 32 pad query columns of each block
        qv = QT.rearrange("p (b c) -> p b c", b=ND)
        nc.vector.memset(qv[:, :, T:TQ], 0.0)

        wv_sb = wpool.tile([P, ND * D], bf16, tag="w", name="wv")
        nc.sync.dma_start(
            out=wv_sb.rearrange("p (b c) -> p b c", b=ND),
            in_=dram_blocks_ap(wv, ND, D),
        )

        # ---- K projection (no bias); KT cols beyond 1030 are 0 ----
        for ec in range(ND):
            psa = psA.tile([P, 512], f32, tag="psA", name="ps_ka")
            psb = psA.tile([P, 512], f32, tag="psA", name="ps_kb")
            for dc in range(ND):
                nc.tensor.matmul(psa, lhsT=wk_sb[:, dc * D + ec * P:dc * D + ec * P + P],
                                 rhs=xsT_all[:, dc * XSW:dc * XSW + 512],
                                 start=(dc == 0), stop=(dc == ND - 1))
            for dc in range(ND):
                nc.tensor.matmul(psb, lhsT=wk_sb[:, dc * D + ec * P:dc * D + ec * P + P],
                                 rhs=xsT_all[:, dc * XSW + 512:dc * XSW + 1024],
                                 start=(dc == 0), stop=(dc == ND - 1))
            pst = psA.tile([P, KW - 1024], f32, tag="psA", name="ps_kt")
            for dc in range(ND):
                nc.tensor.matmul(pst,
                                 lhsT=wk_sb[:, dc * D + ec * P:dc * D + ec * P + P],
                                 rhs=xsT_all[:, dc * XSW + 1024:dc * XSW + KW],
                                 start=(dc == 0), stop=(dc == ND - 1))
            nc.scalar.activation(out=KT[:, ec * KW:ec * KW + 512], in_=psa,
                                 func=Act.Copy)
            nc.vector.tensor_copy(KT[:, ec * KW + 512:ec * KW + 1024], psb)
            nc.scalar.activation(out=KT[:, ec * KW + 1024:ec * KW + KW], in_=pst,
                                 func=Act.Copy)

        wf_sb = wpool.tile([P, ND * D], bf16, tag="w", name="wf")
        nc.sync.dma_start(
            out=wf_sb.rearrange("p (b c) -> p b c", b=ND),
            in_=dram_blocks_ap(wf, ND, D),
        )
        xq_all = big.tile([P, NT * D], bf16, tag="xq")
        nc.sync.dma_start(
            out=xq_all.rearrange("p (b c) -> p b c", b=NT),
            in_=dram_blocks_ap(xq, NT, D),
        )

        # ---- V projection helper: window-aligned chunk tiles (halo rows),
        # emitted interleaved with attention to keep the PE warm ----
        def emit_vproj(ci):
            s = CL * ci
            psa = psA.tile([P, 512], f32, tag="psA", name="ps_va")
            psb = psA.tile([P, 512], f32, tag="psA", name="ps_vb")
            for dc in range(ND):
                nc.tensor.matmul(psa, lhsT=xsT_all[:, dc * XSW + s:dc * XSW + s + P],
                                 rhs=wv_sb[:, dc * D:dc * D + 512],
                                 start=(dc == 0), stop=(dc == ND - 1))
            for dc in range(ND):
                nc.tensor.matmul(psb, lhsT=xsT_all[:, dc * XSW + s:dc * XSW + s + P],
                                 rhs=wv_sb[:, dc * D + 512:dc * D + 1024],
                                 start=(dc == 0), stop=(dc == ND - 1))
            nc.scalar.activation(out=V_all[:, ci * D:ci * D + 512], in_=psa,
                                 func=Act.Copy)
            nc.vector.tensor_copy(V_all[:, ci * D + 512:ci * D + 1024], psb)

        # ---- attention + FC, software-pipelined ----
        # FC chunk c is emitted once PV of its source chunks is emitted.
        fc_at = {2: [0], 3: [1], 4: [2], 6: [3], 7: [4], 8: [5], 10: [6], 12: [7]}

        # slot sl of a group holds head 8g + perm(sl); slots 0-3 (psum bank 0)
        # take the partition-base-0 heads, slots 4-7 (bank 1) the base-64
        # heads: consecutive matmuls into one PSUM bank must share the PE
        # tile row (lhsT partition base) or the exec unit dies.
        def s_head(g, sl):
            return 8 * g + (sl % 4) * 2 + sl // 4

        def emit_scores(ci, g):
            s = CL * ci
            s2 = psS.tile([CL, 1024], f32, tag="psS", name=f"s2_{ci}_{g}")
            for sl in range(8):
                ec = 4 * g + sl % 4
                r = sl // 4
                nc.tensor.matmul(
                    s2[:, sl * P:sl * P + P],
                    lhsT=QT[64 * r:64 * r + 64, ec * TQ + s:ec * TQ + s + CL],
                    rhs=KT[64 * r:64 * r + 64, ec * KW + s:ec * KW + s + P],
                    start=True, stop=True,
                )
            return s2

        def emit_softmax(ci, g, s2):
            pe = pepool.tile([CL, 1024], bf16, tag="pe", name=f"pe_{ci}_{g}")
            nc.scalar.activation(out=pe, in_=s2, func=Act.Exp)
            pev = pe.rearrange("p (h c) -> p h c", h=8)
            nc.gpsimd.tensor_tensor(
                pev, pev,
                band_sb[:, None, :].to_broadcast((CL, 8, P)),
                Alu.mult,
            )
            den = small.tile([CL, 8], f32, tag=f"den{g}", name="den")
            nc.vector.tensor_reduce(out=den, in_=pev,
                                    axis=mybir.AxisListType.X, op=Alu.add)
            rcp = small.tile([CL, 8], f32, tag=f"rcp{g}", name="rcp")
            nc.vector.reciprocal(rcp, den)
            pn = pnpool.tile([CL, 1024], bf16, tag="pn", name=f"pn_{ci}_{g}")
            nc.vector.tensor_tensor(
                pn.rearrange("p (h c) -> p h c", h=8),
                pev,
                rcp[:, :, None].to_broadcast((CL, 8, P)),
                Alu.mult,
            )
            return pn

        def emit_transposes(ci, g, pn):
            pt = psT.tile([P, 1024], bf16, tag="psT", name=f"pt_{ci}_{g}")
            for h in range(8):
                nc.tensor.transpose(pt[:, h * P:h * P + CL],
                                    pn[:, h * P:h * P + P],
                                    idn_sb[0:CL, 0:CL])
            ptsb = ptpool.tile([P, 1024], bf16, tag="pt", name=f"ptsb_{ci}_{g}")
            src = pt.rearrange("p (h c) -> p h c", h=8)[:, :, 0:CL]
            dst = ptsb.rearrange("p (h c) -> p h c", h=8)[:, :, 0:CL]
            if g == 0:
                nc.vector.tensor_copy(dst, src)
            else:
                nc.scalar.activation(out=dst, in_=src, func=Act.Copy)
            return ptsb

        def emit_pv(ci, g, ptsb):
            ot = psO.tile([P, 512], f32, tag="psO", name=f"ot_{ci}_{g}")
            for sl in range(8):
                hh = s_head(g, sl)
                hl = hh - 8 * g
                p_, r = hl // 2, hl % 2
                nc.tensor.matmul(
                    ot[64 * r:64 * r + 64, p_ * P:p_ * P + CL],
                    lhsT=V_all[:, ci * D + hh * DV:ci * D + hh * DV + DV],
                    rhs=ptsb[:, sl * P:sl * P + CL],
                    start=True, stop=True,
                )
            # evict: pair p of this group -> OT block (4g + p), token cols
            s = CL * ci
            ew = CL if ci < NCH - 1 else T - s   # last chunk: only 64 valid
            otv = OT.rearrange("p (b c) -> p b c", b=ND)
            nc.scalar.activation(
                out=otv[:, 4 * g:4 * g + 4, s:s + ew],
                in_=ot.rearrange("p (h c) -> p h c", h=4)[:, :, 0:ew],
                func=Act.Copy,
            )

        def emit_fc(c):
            cs = c * P
            psa = psA.tile([P, 512], f32, tag="psA", name=f"ps_fa{c}")
            psb = psA.tile([P, 512], f32, tag="psA", name=f"ps_fb{c}")
            for ec in range(ND):
                nc.tensor.matmul(psa, lhsT=OT[:, ec * T + cs:ec * T + cs + P],
                                 rhs=wf_sb[:, ec * D:ec * D + 512],
                                 start=(ec == 0), stop=False)
            nc.tensor.matmul(psa, lhsT=idn_sb,
                             rhs=xq_all[:, c * D:c * D + 512],
                             start=False, stop=True)
            for ec in range(ND):
                nc.tensor.matmul(psb, lhsT=OT[:, ec * T + cs:ec * T + cs + P],
                                 rhs=wf_sb[:, ec * D + 512:ec * D + 1024],
                                 start=(ec == 0), stop=False)
            nc.tensor.matmul(psb, lhsT=idn_sb,
                             rhs=xq_all[:, c * D + 512:c * D + 1024],
                             start=False, stop=True)
            # layernorm on psum y = fc + residual
            s0 = lns.tile([P, 1], f32, tag="s0", name="s0")
            s1 = lns.tile([P, 1], f32, tag="s1", name="s1")
            nc.vector.tensor_reduce(out=s0, in_=psa,
                                    axis=mybir.AxisListType.X, op=Alu.add)
            nc.vector.tensor_reduce(out=s1, in_=psb,
                                    axis=mybir.AxisListType.X, op=Alu.add)
            q0 = lns.tile([P, 1], f32, tag="q0", name="q0")
            q1 = lns.tile([P, 1], f32, tag="q1", name="q1")
            ysq = lns.tile([P, 512], f32, tag="ysq", name="ysq")
            nc.scalar.activation(out=ysq, in_=psa, func=Act.Square, accum_out=q0)
            nc.scalar.activation(out=ysq, in_=psb, func=Act.Square, accum_out=q1)
            ysum = lns.tile([P, 1], f32, tag="ysum", name="ysum")
            nc.vector.tensor_add(ysum, s0, s1)
            ssum = lns.tile([P, 1], f32, tag="ssum", name="ssum")
            nc.vector.tensor_add(ssum, q0, q1)
            mean = lns.tile([P, 1], f32, tag="mean", name="mean")
            nc.vector.tensor_scalar_mul(mean, ysum, 1.0 / D)
            msq = lns.tile([P, 1], f32, tag="msq", name="msq")
            nc.vector.tensor_mul(msq, mean, mean)
            var = lns.tile([P, 1], f32, tag="var", name="var")
            nc.vector.scalar_tensor_tensor(
                out=var, in0=ssum, scalar=1.0 / D, in1=msq,
                op0=Alu.mult, op1=Alu.subtract,
            )
            lv = lns.tile([P, 1], f32, tag="lv", name="lv")
            nc.scalar.activation(out=lv, in_=var, func=Act.Ln, bias=eps_sb)
            rstd = lns.tile([P, 1], f32, tag="rstd", name="rstd")
            nc.scalar.activation(out=rstd, in_=lv, func=Act.Exp, scale=-0.5)
            bact = lns.tile([P, 1], f32, tag="bact", name="bact")
            nc.vector.scalar_tensor_tensor(
                out=bact, in0=mean, scalar=-1.0, in1=rstd,
                op0=Alu.mult, op1=Alu.mult,
            )
            y = ypool.tile([P, D], f32, tag="y", name=f"y{c}")
            nc.scalar.activation(out=y[:, 0:512], in_=psa, func=Act.Identity,
                                 bias=bact, scale=rstd)
            nc.scalar.activation(out=y[:, 512:1024], in_=psb, func=Act.Identity,
                                 bias=bact, scale=rstd)
            if apply_affine:
                nc.vector.tensor_mul(y, y, g_bc)
                nc.vector.tensor_add(y, y, b_bc)
            nc.sync.dma_start(out=yo[cs:cs + P, :], in_=y)

        emit_vproj(0)
        emit_vproj(1)
        pn_prev = [None, None]
        for ci in range(13):
            if ci < NCH:
                s2a = emit_scores(ci, 0)
            if 1 <= ci <= NCH:
                pta = emit_transposes(ci - 1, 0, pn_prev[0])
            if ci < NCH:
                s2b = emit_scores(ci, 1)
            if 1 <= ci <= NCH:
                ptb = emit_transposes(ci - 1, 1, pn_prev[1])
            if ci + 2 < NCH:
                emit_vproj(ci + 2)
            if 1 <= ci <= NCH:
                emit_pv(ci - 1, 0, pta)
                emit_pv(ci - 1, 1, ptb)
            if ci < NCH:
                pn_prev[0] = emit_softmax(ci, 0, s2a)
                pn_prev[1] = emit_softmax(ci, 1, s2b)
            for c in fc_at.get(ci, []):
                emit_fc(c)

    nc.compile()
    return nc


def _get_program(apply_affine: bool):
    key = ("prog", apply_affine)
    if key not in _CACHE:
        _CACHE[key] = _build_program(apply_affine)
    return _CACHE[key]


def _host_prep(inputs):
    x = np.asarray(inputs["x"], np.float32)
    xs = np.asarray(inputs["xs"], np.float32)
    w_qs = np.asarray(inputs["w_qs"], np.float32)
    b_qs = np.asarray(inputs["b_qs"], np.float32)
    w_ks = np.asarray(inputs["w_ks"], np.float32)
    w_vs = np.asarray(inputs["w_vs"], np.float32)
    b_vs = np.asarray(inputs["b_vs"], np.float32)
    w_fc = np.asarray(inputs["w_fc"], np.float32)
    b_fc = np.asarray(inputs["b_fc"], np.float32)
    ln_g = np.asarray(inputs["ln_g"], np.float32)
    ln_b = np.asarray(inputs["ln_b"], np.float32)

    apply_affine = not (np.all(ln_g == 1.0) and np.all(ln_b == 0.0))

    bprime = (b_vs @ w_fc + b_fc).astype(np.float32)

    band = np.zeros((CL, P), np.float32)
    for t in range(CL):
        band[t, t:t + 2 * NEI + 1] = 1.0

    shared = {
        "wq": np.ascontiguousarray(w_qs.astype(BF16)),
        "wk": np.ascontiguousarray(w_ks.astype(BF16)),
        "wv": np.ascontiguousarray(w_vs.astype(BF16)),
        "wf": np.ascontiguousarray(w_fc.astype(BF16)),
        "bq": np.ascontiguousarray(
            (b_qs / TEMP).reshape(ND, P).T.astype(np.float32)),
        "band": np.ascontiguousarray(band.astype(BF16)),
        "idn": np.eye(P, dtype=BF16),
    }
    if apply_affine:
        shared["lng"] = np.ascontiguousarray(ln_g.reshape(1, D))
        shared["lnb"] = np.ascontiguousarray(ln_b.reshape(1, D))

    in_maps = []
    half_n = S // 2  # 1024
    for core in range(NCORES):
        b, half = core // 2, core % 2
        t0 = half * half_n
        xc = x[b, t0:t0 + half_n]
        halo = np.zeros((TH, D), np.float32)
        lo = max(0, t0 - NEI)
        hi = min(S, t0 + half_n + NEI)
        halo[lo - (t0 - NEI):hi - (t0 - NEI)] = xs[b, lo:hi]
        m = dict(shared)
        m["xq"] = np.ascontiguousarray((xc + bprime).astype(BF16))
        m["xqT"] = np.ascontiguousarray(xc.T.astype(BF16))
        m["xsT"] = np.ascontiguousarray(halo.T.astype(BF16))
        in_maps.append(m)
    return in_maps, apply_affine


def _run(inputs, trace=False, trace_kwargs=None):
    from concourse.bass_utils import run_bass_kernel_spmd

    in_maps, apply_affine = _host_prep(inputs)
    nc = _get_program(apply_affine)
    res = run_bass_kernel_spmd(
        nc, in_maps, list(range(NCORES)),
        trace=trace, **(trace_kwargs or {})
    )
    y = np.empty((B, S, D), np.float32)
    half_n = S // 2
    for core in range(NCORES):
        b, half = core // 2, core % 2
        y[b, half * half_n:(half + 1) * half_n] = res.results[core]["yo"]
    return y, res


def kernel(**inputs):
    y, _ = _run(inputs)
    return y
